# revision 2
# baseline (speedup 1.0000x reference)
"""Trainium2 Bass kernel for nn_Decoder_40338332844507.

Computes logits = einsum('btc,wpc->bptw', q, W) + b.T[None,:,None,:]
with q [32, 2048, 256] f32, W [49, 32, 256] f32, b [49, 32] f32,
output [32, 32, 2048, 49] f32.

Strategy: data-parallel over batch across 8 NeuronCores (4 batches per
core). Per core, for each 128-token tile the TensorEngine computes
out[t, (p,w)] = qT_tile.T @ Wr in bf16. The device stores the logits
in bf16 (halves the dominant HBM store stream vs f32; rel err stays
~5e-3, well under the 2e-2 gate); the host upcasts to f32 and fuses
the bias add into the upcast. PSUM->SBUF eviction is a pure copy
alternating between the Vector (DVE) and Scalar (Act) engines per
token-tile so neither engine gates the TensorEngine. Token tiles are
strided (t = tp*TL + tl, partition dim = tp) so each output store
covers contiguous DRAM runs of 16*49*2 bytes. The first batch is
computed in p-halves so stores start early; the last batch ends in
p-quarters so the final un-overlapped store tail is only ~1.6 MB.
"""

import json
import sys
import numpy as np
from contextlib import ExitStack

if "/opt/trn_rl_repo" not in sys.path:
    sys.path.insert(0, "/opt/trn_rl_repo")

import concourse.bass as bass
import concourse.tile as tile
from concourse import mybir
from concourse.bass_utils import run_bass_kernel_spmd

B, T, C = 32, 2048, 256
P, WW = 32, 49
N = P * WW  # 1568
N_CORES = 8
B_LOC = B // N_CORES  # 4 batches per core
TL = 16  # token interleave: t = tp*16 + tl -> store runs of 16*49*2 B


def _patch_split_sync_waits():
    """The walrus build on this image accepts at most ONE sync-wait per
    instruction ("Too many sync wait commands" otherwise). Tile emits
    instructions with several waits. Post-process the serialized BIR:
    hoist all but the last wait of each instruction onto 1-wait NoOps
    inserted immediately before it on the same engine (engines execute
    their instruction stream in order, so the semantics are identical)."""
    if getattr(bass.Bass, "_split_waits_patched", False):
        return
    orig = bass.Bass.to_json_bytes

    def to_json_bytes(self):
        m = json.loads(orig(self))
        ctr = 0
        for f in m.get("functions", []):
            for bb in f.get("blocks", []):
                out = []
                for inst in bb.get("instructions", []):
                    si = inst.get("sync_info")
                    if si:
                        waits = si.get("on_wait") or []
                        if len(waits) > 1:
                            for wt in waits[:-1]:
                                ctr += 1
                                nop = {
                                    "engine": inst["engine"],
                                    "ins": [],
                                    "outs": [],
                                    "name": f"I-npw{ctr}",
                                    "opcode": "NoOp",
                                    "sync_info": {"on_wait": [wt], "on_update": []},
                                }
                                if inst.get("debug") is not None:
                                    nop["debug"] = inst["debug"]
                                out.append(nop)
                            si["on_wait"] = waits[-1:]
                    out.append(inst)
                bb["instructions"] = out
        return json.dumps(m).encode()

    bass.Bass.to_json_bytes = to_json_bytes
    bass.Bass._split_waits_patched = True


def build_bass():
    _patch_split_sync_waits()
    nc = bass.Bass("TRN2", target_bir_lowering=False, debug=False)
    f32 = mybir.dt.float32
    bf16 = mybir.dt.bfloat16

    qt = nc.dram_tensor("qt", [B_LOC, C, T], bf16, kind="ExternalInput")
    wr = nc.dram_tensor("wr", [C, N], bf16, kind="ExternalInput")
    o = nc.dram_tensor("o", [B_LOC, P, T, WW], bf16, kind="ExternalOutput")

    with tile.TileContext(nc) as tc:
        with ExitStack() as ctx:
            consts = ctx.enter_context(tc.tile_pool(name="consts", bufs=1))
            qpool = ctx.enter_context(tc.tile_pool(name="qpool", bufs=2))
            opool = ctx.enter_context(tc.tile_pool(name="opool", bufs=2))
            psum = ctx.enter_context(tc.tile_pool(name="psum", bufs=2, space="PSUM"))

            wr_sb = [
                consts.tile([128, N], bf16, tag=f"wr{k}", name=f"wr{k}")
                for k in range(2)
            ]
            nc.sync.dma_start(wr_sb[0][:], wr.ap()[0:128, :])
            nc.scalar.dma_start(wr_sb[1][:], wr.ap()[128:256, :])

            state = {"ev": 0, "st": 0}

            def unit(b, q_v, oh, p0, np_, uname):
                """Compute o[b, p0:p0+np_, :, :] (all tokens) and store it."""
                nw = np_ * WW
                for tl in range(TL):
                    pt = psum.tile([128, 2048], f32, tag="pt", name=f"pt_{uname}_{tl}")
                    for k in range(2):
                        for n0 in range(0, nw, 512):
                            n1 = min(n0 + 512, nw)
                            nc.tensor.matmul(
                                pt[:, n0:n1],
                                q_v[k][:, tl, :],
                                wr_sb[k][:, p0 * WW + n0 : p0 * WW + n1],
                                start=(k == 0),
                                stop=(k == 1),
                            )
                    pv = pt[:, :nw].rearrange("t (p w) -> t p w", w=WW)
                    dst = oh[:, p0 : p0 + np_, bass.ds(tl * WW, WW)]
                    if state["ev"] % 2 == 0:
                        nc.vector.tensor_copy(dst, pv[:])
                    else:
                        nc.scalar.copy(dst, pv[:])
                    state["ev"] += 1
                # store in <=16-head chunks, alternating the two HWDGE queues
                for ps in range(p0, p0 + np_, 16):
                    pe = min(ps + 16, p0 + np_)
                    eng = (nc.sync, nc.scalar)[state["st"] % 2]
                    state["st"] += 1
                    d = (
                        o.ap()[b, ps:pe, :, :]
                        .rearrange("p (t l) w -> t p (l w)", l=TL)
                    )
                    eng.dma_start(d, oh[:, ps:pe, :])

            for b in range(B_LOC):
                # load q[b] transposed: two [128(c), 2048(t)] bf16 tiles
                q_sb = [
                    qpool.tile([128, T], bf16, tag=f"q{k}", name=f"q{k}_{b}")
                    for k in range(2)
                ]
                nc.gpsimd.dma_start(q_sb[0][:], qt.ap()[b, 0:128, :])
                nc.gpsimd.dma_start(q_sb[1][:], qt.ap()[b, 128:256, :])
                # t split as (tp, tl); lhsT tiles are [c, tp] (stride TL)
                q_v = [
                    q_sb[k][:].rearrange("c (p l) -> c l p", l=TL) for k in range(2)
                ]
                oh = opool.tile([128, P, TL * WW], bf16, tag="oh", name=f"oh{b}")

                if b == 0:
                    # p-halves: first store leaves at ~half-batch mark
                    unit(b, q_v, oh, 0, 16, "b0h0")
                    unit(b, q_v, oh, 16, 16, "b0h1")
                elif b == B_LOC - 1:
                    # shrink the final un-overlapped store tail
                    unit(b, q_v, oh, 0, 16, "b3h0")
                    unit(b, q_v, oh, 16, 8, "b3q2")
                    unit(b, q_v, oh, 24, 8, "b3q3")
                else:
                    unit(b, q_v, oh, 0, 32, f"b{b}")
    return nc


_NC_CACHE = None


def _get_nc():
    global _NC_CACHE
    if _NC_CACHE is None:
        _NC_CACHE = build_bass()
    return _NC_CACHE


def prep_core_inputs(q, W):
    """Host-side layout prep: activation transpose + weight packing, bf16."""
    import ml_dtypes

    bf = ml_dtypes.bfloat16
    q = np.asarray(q, dtype=np.float32)
    Wt = np.asarray(W, dtype=np.float32)
    qt = np.ascontiguousarray(q.transpose(0, 2, 1).astype(bf))  # [B, C, T]
    wr = np.ascontiguousarray(Wt.transpose(2, 1, 0).reshape(C, N).astype(bf))
    return [
        {"qt": qt[c * B_LOC : (c + 1) * B_LOC], "wr": wr}
        for c in range(N_CORES)
    ]


def assemble_output(res, bvec):
    """Gather per-core bf16 logits, upcast to f32 and fuse the bias add."""
    bias = np.asarray(bvec, dtype=np.float32).T[None, :, None, :]  # [1,P,1,W]
    out = np.empty((B, P, T, WW), dtype=np.float32)
    for c in range(N_CORES):
        sl = slice(c * B_LOC, (c + 1) * B_LOC)
        out[sl] = res.results[c]["o"].astype(np.float32)
        out[sl] += bias
    return out


def kernel(q, W, b):
    nc = _get_nc()
    in_maps = prep_core_inputs(q, W)
    res = run_bass_kernel_spmd(nc, in_maps, core_ids=list(range(N_CORES)))
    return assemble_output(res, b)
